# revision 71
# baseline (speedup 1.0000x reference)
"""Multi-head attention (B=8, N=1024, C=768, H=12, D=64) on 8 TRN2 NeuronCores.

Strategy: pure data-parallel over batch (B == n_cores == 8), no collectives.
Each core computes full 12-head attention for one batch element in a fully
transposed layout (channels on SBUF partitions).

v17 design (vs. v2 @ 211us -> ~193us):
  - Lead-in: weight loads split by first-needed slice across the DMA
    queues (x->sync, wv/wq->scalar, wk->gpsimd, cot-0 slices first); a PE
    pre-warm burst on a memset scratch lifts the HAM clock gate toward
    8/8 before the first real matmul; all of v(heads 0:8) moves pre-pair.
  - Soft pair boundaries: each pair's odd-head PV block + evacuation is
    deferred into the next pair, split 8/8 around the next pair's first
    exp, so ACT never bubbles at a pair boundary waiting for QK j1
    behind a 16-matmul block.
  - All norm work runs on DVE/sync, never as gpsimd compute: gpsimd
    tensor ops both lock the shared SBUF port pair and starve SWDGE DMA
    descriptor generation, backing up the evac DMAs -> vstag pool ->
    DVE -> PE fill-slot cascade.  vstag depth 8 decouples DVE from the
    gpsimd evacuation queue.
  - Normalization: three batched reciprocal chains (load split from the
    DVE reciprocal so it never head-of-line blocks the DVE queue; sums
    reload on the scalar queue past the sync queue's exp(bias) backlog);
    ALL rba broadcasts ([64,N] per head, half the bytes) and all twelve
    norm-multiplies are deferred to the post-pair window where sync/DVE
    are idle, overlapping the output-projection accumulation.
  - Output is stored bf16 (cast to f32 on host): halves the store drain;
    bf16 rounding of the final result is well inside the error budget.
  - pe_warm only where the PE is otherwise filler-starved (pair 5).

Core attention layout (unchanged from v2):
  - Heads processed in PAIRS; even head's K/Q on SBUF partitions 0:64, odd
    on 64:128 so the two QK^T matmuls run concurrently via PE row tiling.
  - S-pair tiles [128, 1024] f32 (2 PSUM banks), one FD=1024 ACT exp each;
    additive bias applied as exp(S/8)*exp(bias) with exp(bias) precomputed
    on host, multiplied on DVE at 2x bf16 rate.
  - PV keeps the ones-column trick (row 64 = softmax sum); odd head's PV
    runs as a dense 16-matmul block at pair end on SBUF-buffered pt tiles.
"""

import os
import sys
import numpy as np

for _p in ("/opt/trn_rl_repo", "/root/.axon_site/_ro/trn_rl_repo"):
    if os.path.isdir(_p) and _p not in sys.path:
        sys.path.append(_p)

import ml_dtypes

BF16 = ml_dtypes.bfloat16

B, N, C = 8, 1024, 768
H, D = 12, 64
CT = C // 128         # 6 channel tiles
NT = N // 128         # 8 key tiles
F = 512
NP = H // 2           # 6 head pairs

_cache = {}


def _build():
    import concourse.bass as bass
    import concourse.tile as tile
    from concourse import bacc, mybir

    f32 = mybir.dt.float32
    bf16 = mybir.dt.bfloat16
    AF = mybir.ActivationFunctionType
    ALU = mybir.AluOpType

    nc = bacc.Bacc("TRN2", target_bir_lowering=False)

    # x and weights arrive host-packed as [partition, ci, free] so the
    # whole-tensor loads are contiguous on both sides (max-size DMA
    # packets, minimal descriptor work in the critical lead-in)
    xT_d = nc.dram_tensor("xT", [128, CT, N], bf16, kind="ExternalInput")
    wqT_d = nc.dram_tensor("wqT", [128, CT, C], bf16, kind="ExternalInput")
    wkT_d = nc.dram_tensor("wkT", [128, CT, C], bf16, kind="ExternalInput")
    wvT_d = nc.dram_tensor("wvT", [128, CT, C], bf16, kind="ExternalInput")
    wpT_d = nc.dram_tensor("wpT", [128, CT, C], bf16, kind="ExternalInput")
    bpT_d = nc.dram_tensor("bpT", [128, CT], f32, kind="ExternalInput")
    # exp(attn_bias) packed per (pair, key-tile j, query-half nb):
    # [...,0:512] = even head, [...,512:1024] = odd head
    eb_d = nc.dram_tensor("ebPk", [NP, NT, 2, 128, 2 * F], bf16,
                          kind="ExternalInput")
    outT_d = nc.dram_tensor("outT", [C, N], bf16, kind="ExternalOutput")
    # softmax-sum scratch per normalization batch
    HA = 8
    sA_scr = nc.dram_tensor("sA_scr", [HA * N], bf16)
    sB1_scr = nc.dram_tensor("sB1_scr", [2 * N], bf16)
    sB2_scr = nc.dram_tensor("sB2_scr", [2 * N], bf16)
    rA_scr = nc.dram_tensor("rA_scr", [1, HA * N], bf16)
    rB1_scr = nc.dram_tensor("rB1_scr", [1, 2 * N], bf16)
    rB2_scr = nc.dram_tensor("rB2_scr", [1, 2 * N], bf16)

    with tile.TileContext(nc) as tc:
        with tc.tile_pool(name="persist", bufs=1) as pers:
            xTb = pers.tile([128, CT, N], bf16, tag="xT")
            wqb = pers.tile([128, CT, C], bf16, tag="wq")
            wkb = pers.tile([128, CT, C], bf16, tag="wk")
            wvb = pers.tile([128, CT, C], bf16, tag="wv")
            wpb = pers.tile([128, CT, C], bf16, tag="wp")
            bpb = pers.tile([128, CT], f32, tag="bp")
            # row 64 collects softmax sums (same partition as pv row 64)
            rba = pers.tile([128, H * N], bf16, tag="rba")
            qtb = pers.tile([128, CT, N], bf16, tag="qt")
            ktb = pers.tile([128, CT, N], bf16, tag="kt")
            vb = pers.tile([128, NT, H, D + 1], bf16, tag="v")
            atb = pers.tile([128, CT, N], bf16, tag="at")
            wsrc = pers.tile([128, 384], bf16, tag="wsrc")

            # fine-grained, need-ordered loads: the lead-in's ~5MB is
            # aggregate-bandwidth-bound, and pair 0's first QK needs ALL
            # of x (both query halves), so x gets the head of BOTH the
            # sync and scalar queues; weights follow in first-use order
            # x gates the first QK projection and the DMA engines are
            # shared round-robin across queues, so x rides the head of
            # ALL THREE queues; weights follow in first-use order
            nc.sync.dma_start(xTb[:, :, 0:256], xT_d[:, :, 0:256])
            nc.scalar.dma_start(xTb[:, :, 256:512], xT_d[:, :, 256:512])
            nc.gpsimd.dma_start(xTb[:, :, 512:768], xT_d[:, :, 512:768])
            nc.sync.dma_start(xTb[:, :, 768:1024], xT_d[:, :, 768:1024])
            nc.gpsimd.dma_start(wkb[:, :, 0:128], wkT_d[:, :, 0:128])
            nc.scalar.dma_start(wqb[:, :, 0:128], wqT_d[:, :, 0:128])
            nc.gpsimd.dma_start(wkb[:, :, 128:C], wkT_d[:, :, 128:C])
            nc.scalar.dma_start(wvb[:, :, 0:512], wvT_d[:, :, 0:512])
            nc.scalar.dma_start(wqb[:, :, 128:C], wqT_d[:, :, 128:C])
            nc.scalar.dma_start(wvb[:, :, 512:C], wvT_d[:, :, 512:C])
            nc.scalar.dma_start(bpb, bpT_d[:])
            nc.scalar.dma_start(wpb, wpT_d[:])

            nc.vector.memset(wsrc, 1.0)
            nc.vector.memset(vb[:, :, :, D:D + 1], 1.0)

            with tc.tile_pool(name="ups", bufs=2, space="PSUM") as pU, \
                 tc.tile_pool(name="pvps", bufs=2, space="PSUM") as pPV, \
                 tc.tile_pool(name="fillps", bufs=2, space="PSUM") as pF, \
                 tc.tile_pool(name="ebb", bufs=4) as ebp, \
                 tc.tile_pool(name="vstagb", bufs=8) as vstagp, \
                 tc.tile_pool(name="nrmb", bufs=1) as nrm, \
                 tc.tile_pool(name="ptb", bufs=18) as ptp:

                eb_tiles = {}

                def eb_load(pr, j):
                    ebt = ebp.tile([128, 2, 2 * F], bf16, tag="eb")
                    nc.sync.dma_start(
                        ebt, eb_d[pr, j].rearrange("nb p q -> p nb q"))
                    eb_tiles[(pr, j)] = ebt

                def pe_prewarm(n):
                    """Dense burst of matmuls on the memset scratch: keeps
                    the PE busy (HAM clock gate at 8/8) from boot until x
                    lands (~17us) -- any >3.4us idle gap here re-throttles
                    the clock and the whole pre-pair runs at 1.2GHz."""
                    for _ in range(n):
                        ps = pF.tile([128, F], f32, tag="fill",
                                     name="prewarm")
                        nc.tensor.matmul(
                            ps[:, 0:384],
                            lhsT=wsrc[:, 0:128],
                            rhs=wsrc[:, 0:384],
                            start=True, stop=True,
                        )

                def v_proj(h0, nh, nt):
                    """V projection for heads [h0, h0+nh) at key-tile nt."""
                    f0, fw = h0 * D, nh * D
                    ps = pF.tile([128, F], f32, tag="fill")
                    for ci in range(CT):
                        nc.tensor.matmul(
                            ps[:, :fw],
                            lhsT=xTb[:, ci, nt * 128:(nt + 1) * 128],
                            rhs=wvb[:, ci, f0:f0 + fw],
                            start=(ci == 0),
                            stop=(ci == CT - 1),
                        )
                    nc.vector.tensor_copy(
                        vb[:, nt, h0:h0 + nh, 0:D],
                        ps[:, :fw].rearrange("p (h d) -> p h d", d=D),
                    )

                def pe_warm():
                    """Redundant 6-matmul group (recomputes k-projection
                    tile 0 into a dead PSUM tile, never read).  Emitted in
                    filler-starved stretches so the PE's activity monitor
                    does not re-throttle the clock (K=4/8) on micro-idle."""
                    ps = pF.tile([128, F], f32, tag="fill", name="warm")
                    for ci in range(CT):
                        nc.tensor.matmul(
                            ps,
                            lhsT=wkb[:, ci, 0:128],
                            rhs=xTb[:, ci, 0:F],
                            start=(ci == 0),
                            stop=(ci == CT - 1),
                        )

                def kq_sub(which, cot, nb):
                    wb, dst = (wkb, ktb) if which == "k" else (wqb, qtb)
                    ps = pF.tile([128, F], f32, tag="fill")
                    for ci in range(CT):
                        nc.tensor.matmul(
                            ps,
                            lhsT=wb[:, ci, cot * 128:(cot + 1) * 128],
                            rhs=xTb[:, ci, nb * F:(nb + 1) * F],
                            start=(ci == 0),
                            stop=(ci == CT - 1),
                        )
                    nc.vector.tensor_copy(
                        dst[:, cot, nb * F:(nb + 1) * F], ps)

                def kq_ct(cot):
                    for which in ("k", "q"):
                        for nb in range(2):
                            kq_sub(which, cot, nb)

                def qk_pair(pr, j, nb):
                    """Row-tiled QK^T for both heads of the pair: even head
                    on PE rows 0:64, odd head on rows 64:128, concurrent."""
                    ksl = slice(j * 128, (j + 1) * 128)
                    qsl = slice(nb * F, (nb + 1) * F)
                    sp = pU.tile([128, 2 * F], f32, tag="ps",
                                 name=f"sp_{pr}_{j}_{nb}")
                    nc.tensor.matmul(
                        sp[:, 0:F],
                        lhsT=ktb[0:64, pr, ksl],
                        rhs=qtb[0:64, pr, qsl],
                        start=True, stop=True,
                    )
                    nc.tensor.matmul(
                        sp[:, F:2 * F],
                        lhsT=ktb[64:128, pr, ksl],
                        rhs=qtb[64:128, pr, qsl],
                        start=True, stop=True,
                    )
                    return sp

                def pv_evac(pr, hl, pv_nb):
                    """Evacuate one head's two PV chains; softmax-sum row
                    64 goes straight to the DRAM scratch its norm batch
                    reads."""
                    h = 2 * pr + hl
                    s_scr, hh = (
                        (sA_scr, h) if h < HA
                        else (sB1_scr, h - HA) if h < HA + 2
                        else (sB2_scr, h - HA - 2))
                    for nb in range(2):
                        qsl = slice(nb * F, (nb + 1) * F)
                        vstag = vstagp.tile([D + 1, F], bf16, tag="vstag")
                        nc.vector.tensor_copy(vstag, pv_nb[nb])
                        nc.gpsimd.dma_start(
                            atb[64 * hl:64 * hl + 64, pr, qsl],
                            vstag[0:D, :])
                        nc.gpsimd.dma_start(
                            s_scr[hh * N + nb * F:hh * N + (nb + 1) * F],
                            vstag[D:D + 1, :])

                def attn_pair(pr, fillers, prev_tail=None):
                    """Attention for heads (2*pr, 2*pr+1).

                    Emission order per j: exp/mul for j, then QK for j+1
                    (so the PE never head-of-line blocks on the softmax
                    chain), then the even head's PV matmuls for j, then
                    fillers.  The odd head's PV chains run as one dense
                    16-matmul block on the SBUF-buffered pt tiles --
                    DEFERRED into the start of the NEXT pair (returned as
                    a tail closure, emitted right after the next pair's
                    first QK), so the next pair's exp stream starts on ACT
                    while the PE runs the block: the pair-boundary ACT
                    bubble (~3us) disappears.
                    """
                    sps = [qk_pair(pr, 0, nb) for nb in range(2)]
                    if prev_tail is not None:
                        prev_tail[0]()
                    pv0 = [pPV.tile([D + 1, F], f32, tag="pv",
                                    name=f"pv_{pr}_0_{nb}")
                           for nb in range(2)]
                    pts_all = []
                    for j in range(NT):
                        ebt = eb_tiles.pop((pr, j))
                        pts = []
                        for nb in range(2):
                            pt = ptp.tile([128, 2 * F], bf16, tag="pt",
                                          name=f"pt_{pr}_{j}_{nb}")
                            nc.scalar.activation(
                                pt, sps[nb], AF.Exp, scale=0.125)
                            nc.vector.tensor_tensor(
                                pt, pt, ebt[:, nb, :], ALU.mult)
                            pts.append(pt)
                        pts_all.append(pts)
                        if j + 1 < NT:
                            sps = [qk_pair(pr, j + 1, nb) for nb in range(2)]
                        if j == 0 and prev_tail is not None:
                            prev_tail[1]()
                        for nb in range(2):
                            nc.tensor.matmul(
                                pv0[nb],
                                lhsT=vb[:, j, 2 * pr, :],
                                rhs=pts[nb][:, 0:F],
                                start=(j == 0),
                                stop=(j == NT - 1),
                            )
                        for fn in fillers.get(j, ()):
                            fn()

                    pv1 = []

                    def tail1():
                        pv_evac(pr, 0, pv0)
                        pv1.extend(
                            pPV.tile([D + 1, F], f32, tag="pv",
                                     name=f"pv_{pr}_1_{nb}")
                            for nb in range(2))
                        for j in range(NT // 2):
                            for nb in range(2):
                                nc.tensor.matmul(
                                    pv1[nb],
                                    lhsT=vb[:, j, 2 * pr + 1, :],
                                    rhs=pts_all[j][nb][:, F:2 * F],
                                    start=(j == 0),
                                    stop=False,
                                )

                    def tail2():
                        for j in range(NT // 2, NT):
                            for nb in range(2):
                                nc.tensor.matmul(
                                    pv1[nb],
                                    lhsT=vb[:, j, 2 * pr + 1, :],
                                    rhs=pts_all[j][nb][:, F:2 * F],
                                    start=False,
                                    stop=(j == NT - 1),
                                )
                        pv_evac(pr, 1, pv1)
                    return tail1, tail2

                _NB = [
                    (0, HA, sA_scr, rA_scr),
                    (HA, 2, sB1_scr, rB1_scr),
                    (HA + 2, 2, sB2_scr, rB2_scr),
                ]
                nb_sb = {}

                def norm_load(batch):
                    """Reload the batch's softmax sums.  On the scalar
                    queue: three small issues total, so it neither waits
                    behind the sync queue's exp(bias) backlog nor stalls
                    the gpsimd evacuation stream."""
                    h0, nh, s_scr, r_scr = _NB[batch]
                    cols = nh * N // 128
                    sb = nrm.tile([128, HA * N // 128], bf16, tag="sb")
                    nc.scalar.dma_start(
                        sb[:, :cols],
                        s_scr[:].rearrange("(p f) -> p f", p=128))
                    nb_sb[batch] = sb

                def norm_fin(batch):
                    """Reciprocal + store + per-head broadcasts.  Emitted
                    a few filler slots after norm_load so the DVE
                    reciprocal never head-of-line blocks the DVE queue
                    waiting for the sums to land."""
                    h0, nh, s_scr, r_scr = _NB[batch]
                    cols = nh * N // 128
                    sb = nb_sb.pop(batch)
                    rc32 = nrm.tile([128, HA * N // 128], f32, tag="rc32")
                    nc.vector.reciprocal(rc32[:, :cols], sb[:, :cols])
                    rcb = nrm.tile([128, HA * N // 128], bf16, tag="rcb")
                    nc.vector.tensor_copy(rcb[:, :cols], rc32[:, :cols])
                    nc.sync.dma_start(
                        r_scr[0, :].rearrange("(p f) -> p f", p=128),
                        rcb[:, :cols])

                def norm_bcast(h, eng=None):
                    """Broadcast head h's reciprocals into the 64
                    partitions its norm-multiply reads.  Emitted in the
                    post-pair window (sync queue is idle there) -- doing
                    this mid-pair delayed the exp(bias) prefetch by the
                    ~1MB of broadcast traffic and stretched pairs 4-5."""
                    for b0, nh, _, r_scr in _NB:
                        if b0 <= h < b0 + nh:
                            break
                    po = 64 * (h % 2)
                    (eng or nc.sync).dma_start(
                        rba[po:po + 64, h * N:(h + 1) * N],
                        r_scr[:, (h - b0) * N:(h - b0 + 1) * N]
                        .to_broadcast([64, N]))

                def norm_mul(h, engine=None):
                    """Normalize-multiply one head's atb slice.  Always on
                    DVE: gpsimd compute both locks the shared SBUF port
                    pair and starves its own DMA descriptor generation,
                    backing up the evacuation queue."""
                    ct, po = h // 2, 64 * (h % 2)
                    sl = atb[po:po + 64, ct, :]
                    eng = engine or nc.vector
                    eng.tensor_tensor(
                        sl, sl, rba[po:po + 64, h * N:(h + 1) * N], ALU.mult)

                # ---- emission schedule --------------------------------
                EB_DEPTH = 4
                for t in range(EB_DEPTH):
                    eb_load(0, t)
                pe_prewarm(26)
                kq_ct(0)
                for nt in range(NT):
                    v_proj(0, 8, nt)

                # fillers per pair: kq projection for pair+1 (incl. pair
                # 1's k inside pair 0), rolling EB prefetch, V projection
                # for heads 8:12 during pairs 2-3, per-pair normalization
                # one pair behind, PE warm-keepers in the filler-starved
                # pairs 4-5.
                prev_tail = None
                for pr in range(NP):
                    fill = {j: [] for j in range(NT)}
                    for j in range(NT):
                        t = pr * NT + j + EB_DEPTH
                        if t < NP * NT:
                            fill[j].append(
                                lambda a=t // NT, b=t % NT: eb_load(a, b))
                    nxt = pr + 1
                    if nxt < NP:
                        fill[1].append(lambda n=nxt: kq_sub("k", n, 0))
                        fill[2].append(lambda n=nxt: kq_sub("k", n, 1))
                        fill[4].append(lambda n=nxt: kq_sub("q", n, 0))
                        fill[5].append(lambda n=nxt: kq_sub("q", n, 1))
                    if pr == 2:
                        for nt in range(4):
                            fill[6].append(lambda t=nt: v_proj(8, 4, t))
                    if pr == 3:
                        for nt in range(4, NT):
                            fill[6].append(lambda t=nt: v_proj(8, 4, t))
                    if pr == 4:
                        # heads 0..7 sums complete after pair 3's evac
                        # (whose deferred tail runs at this pair's start);
                        # only the cheap reciprocal chains run inside the
                        # pairs -- all norm-multiplies are deferred to the
                        # output-projection phase where DVE is idle
                        fill[0].append(lambda: norm_load(0))
                        fill[2].append(lambda: norm_fin(0))
                    if pr == 5:
                        fill[0].append(lambda: norm_load(1))
                        fill[2].append(lambda: norm_fin(1))
                        fill[3].append(pe_warm)
                        # heads 0-9 reciprocal broadcasts: their data is
                        # ready (batch 0 stored in pair 4, batch 1 by
                        # fill[2]) and pair 5's eb prefetch is done by
                        # j3, so the sync queue is free -- landing them
                        # here lets every norm-multiply fire immediately
                        # at tail start instead of gating the projection
                        for j, hs in ((4, (0, 1, 2)), (5, (3, 4, 5)),
                                      (6, (6, 7)), (7, (8, 9))):
                            for h in hs:
                                fill[j].append(lambda hh=h: norm_bcast(hh))
                    prev_tail = attn_pair(pr, fill, prev_tail)

                # tail pipeline: pair 5's deferred PV block + evac runs
                # on PE while the reciprocal broadcasts stream on the
                # idle sync queue and the norm-multiplies drain on DVE;
                # batch 2's reciprocal is emitted mid-stream so it never
                # head-of-line blocks the DVE queue
                prev_tail[0]()
                prev_tail[1]()
                norm_load(2)
                for h in range(4):
                    norm_mul(h)
                # batch-2's reciprocal is emitted mid-mul-stream: late
                # enough that the DVE reciprocal has its data, early
                # enough that the h10/11 chain doesn't gate the final
                # projection tiles
                norm_fin(2)
                for h in range(4, 10):
                    norm_mul(h)

            # ---- output projection ------------------------------------
            # [128, 512] granularity (one PSUM bank per tile, 8 in
            # flight).  Pass 1 (ci 0..4: heads 0..9, normalized by
            # mid-pair-5) for the first 8 tiles is emitted BEFORE the
            # heads-10/11 normalize-multiplies; each tile then finishes
            # with the ci=5 matmul, a small ACT bias-add, and a store
            # alternating the sync/scalar queues.  Emitting more than 8
            # pass-1 tiles would deadlock the PE FIFO on pool-slot reuse.
            with tc.tile_pool(name="ops", bufs=8, space="PSUM") as pC, \
                 tc.tile_pool(name="otb", bufs=3) as otp:

                def oproj_mm(ps, cot, nb, ci, start):
                    nc.tensor.matmul(
                        ps,
                        lhsT=wpb[:, ci, cot * 128:(cot + 1) * 128],
                        rhs=atb[:, ci, nb * F:(nb + 1) * F],
                        start=start,
                        stop=False,
                    )

                def oproj_fin(psn, cot):
                    """Finish both query halves of one output-channel
                    tile: ci=5 matmuls, bias-add ACTs, ONE merged store
                    (12 -> 6 store issues: the serialized per-store issue
                    cost on the sync queue dominated the output drain)."""
                    ot = otp.tile([128, 2 * F], bf16, tag="ot",
                                  name=f"ot_{cot}")
                    for nb in range(2):
                        nc.tensor.matmul(
                            psn[nb],
                            lhsT=wpb[:, CT - 1, cot * 128:(cot + 1) * 128],
                            rhs=atb[:, CT - 1, nb * F:(nb + 1) * F],
                            start=False, stop=True,
                        )
                    for nb in range(2):
                        nc.scalar.activation(
                            ot[:, nb * F:(nb + 1) * F], psn[nb],
                            AF.Identity, bias=bpb[:, cot:cot + 1])
                    nc.sync.dma_start(
                        outT_d[cot * 128:(cot + 1) * 128, :], ot)

                # pass 1 (ci 0..4, heads 0..9) runs ci-MAJOR so the first
                # 16 matmuls only need heads 0-3 normalized: the tile
                # accumulation pipelines with the norm-multiply stream
                # still draining on DVE
                pss = {(cot, nb): pC.tile([128, F], f32, tag="o",
                                          name=f"po_{cot}_{nb}")
                       for cot in range(4) for nb in range(2)}
                for i, ci in enumerate(range(CT - 1)):
                    for (cot, nb), ps in pss.items():
                        oproj_mm(ps, cot, nb, ci, start=(i == 0))
                norm_bcast(10)
                norm_bcast(11)
                norm_mul(10)
                norm_mul(11)
                for cot in range(4):
                    oproj_fin([pss[(cot, 0)], pss[(cot, 1)]], cot)
                for cot in range(4, CT):
                    psn = [pC.tile([128, F], f32, tag="o",
                                   name=f"po_{cot}_{nb}")
                           for nb in range(2)]
                    for i, ci in enumerate(range(CT - 1)):
                        for nb in range(2):
                            oproj_mm(psn[nb], cot, nb, ci, start=(i == 0))
                    oproj_fin(psn, cot)

    nc.compile()
    return nc


def _get_nc():
    if "nc" not in _cache:
        _cache["nc"] = _build()
    return _cache["nc"]


def _pack(mT):
    """[C, F] -> [128, CT, F] with row ci*128+p landing at [p, ci]."""
    f = mT.shape[1]
    return np.ascontiguousarray(
        mT.reshape(CT, 128, f).transpose(1, 0, 2)).astype(BF16)


def prep_in_maps(x, attn_bias, Wq, Wk, Wv, Wp, bp):
    """Host-side sharding + layout prep (transposes/casts/exp of bias)."""
    wqT = _pack(Wq.T)
    wkT = _pack(Wk.T)
    wvT = _pack(Wv.T)
    wpT = _pack(Wp.T)
    bpT = np.ascontiguousarray(bp.astype(np.float32).reshape(CT, 128).T)
    # exp(bias)^T packed per (pair, key-tile, query-half): see kernel docstr
    E = np.exp(attn_bias[0].astype(np.float32)).transpose(0, 2, 1)
    E = np.ascontiguousarray(E).reshape(H, NT, 128, 2, F)
    ebPk = np.empty((NP, NT, 2, 128, 2 * F), dtype=np.float32)
    for pr in range(NP):
        ebPk[pr, :, :, :, 0:F] = E[2 * pr].transpose(0, 2, 1, 3)
        ebPk[pr, :, :, :, F:2 * F] = E[2 * pr + 1].transpose(0, 2, 1, 3)
    ebPk = ebPk.astype(BF16)
    in_maps = []
    for b in range(B):
        in_maps.append({
            "xT": _pack(np.asarray(x[b]).T),
            "wqT": wqT, "wkT": wkT, "wvT": wvT, "wpT": wpT,
            "bpT": bpT, "ebPk": ebPk,
        })
    return in_maps


def run(in_maps, trace=False, **kw):
    from concourse.bass_utils import run_bass_kernel_spmd

    nc = _get_nc()
    return run_bass_kernel_spmd(
        nc, in_maps, core_ids=list(range(B)), trace=trace, **kw
    )


def kernel(x, attn_bias, Wq, Wk, Wv, Wp, bp):
    res = run(prep_in_maps(x, attn_bias, Wq, Wk, Wv, Wp, bp))
    out = np.stack(
        [res.results[b]["outT"].T for b in range(B)]
    ).astype(np.float32)
    return out


# revision 74
# speedup vs baseline: 1.0651x; 1.0651x over previous
"""Multi-head attention (B=8, N=1024, C=768, H=12, D=64) on 8 TRN2 NeuronCores.

Strategy: pure data-parallel over batch (B == n_cores == 8), no collectives.
Each core computes full 12-head attention for one batch element in a fully
transposed layout (channels on SBUF partitions).

v17 design (vs. v2 @ 211us -> ~193us):
  - Lead-in: weight loads split by first-needed slice across the DMA
    queues (x->sync, wv/wq->scalar, wk->gpsimd, cot-0 slices first); a PE
    pre-warm burst on a memset scratch lifts the HAM clock gate toward
    8/8 before the first real matmul; all of v(heads 0:8) moves pre-pair.
  - Soft pair boundaries: each pair's odd-head PV block + evacuation is
    deferred into the next pair, split 8/8 around the next pair's first
    exp, so ACT never bubbles at a pair boundary waiting for QK j1
    behind a 16-matmul block.
  - All norm work runs on DVE/sync, never as gpsimd compute: gpsimd
    tensor ops both lock the shared SBUF port pair and starve SWDGE DMA
    descriptor generation, backing up the evac DMAs -> vstag pool ->
    DVE -> PE fill-slot cascade.  vstag depth 8 decouples DVE from the
    gpsimd evacuation queue.
  - Normalization: three batched reciprocal chains (load split from the
    DVE reciprocal so it never head-of-line blocks the DVE queue; sums
    reload on the scalar queue past the sync queue's exp(bias) backlog);
    ALL rba broadcasts ([64,N] per head, half the bytes) and all twelve
    norm-multiplies are deferred to the post-pair window where sync/DVE
    are idle, overlapping the output-projection accumulation.
  - Output is stored bf16 (cast to f32 on host): halves the store drain;
    bf16 rounding of the final result is well inside the error budget.
  - pe_warm only where the PE is otherwise filler-starved (pair 5).

Core attention layout (unchanged from v2):
  - Heads processed in PAIRS; even head's K/Q on SBUF partitions 0:64, odd
    on 64:128 so the two QK^T matmuls run concurrently via PE row tiling.
  - S-pair tiles [128, 1024] f32 (2 PSUM banks), one FD=1024 ACT exp each;
    additive bias applied as exp(S/8)*exp(bias) with exp(bias) precomputed
    on host, multiplied on DVE at 2x bf16 rate.
  - PV keeps the ones-column trick (row 64 = softmax sum); odd head's PV
    runs as a dense 16-matmul block at pair end on SBUF-buffered pt tiles.
"""

import os
import sys
import numpy as np

for _p in ("/opt/trn_rl_repo", "/root/.axon_site/_ro/trn_rl_repo"):
    if os.path.isdir(_p) and _p not in sys.path:
        sys.path.append(_p)

import ml_dtypes

BF16 = ml_dtypes.bfloat16

B, N, C = 8, 1024, 768
H, D = 12, 64
CT = C // 128         # 6 channel tiles
NT = N // 128         # 8 key tiles
F = 512
NP = H // 2           # 6 head pairs

_cache = {}


def _build():
    import concourse.bass as bass
    import concourse.tile as tile
    from concourse import bacc, mybir

    f32 = mybir.dt.float32
    bf16 = mybir.dt.bfloat16
    AF = mybir.ActivationFunctionType
    ALU = mybir.AluOpType

    nc = bacc.Bacc("TRN2", target_bir_lowering=False)

    # x and weights arrive host-packed as [partition, ci, free] so the
    # whole-tensor loads are contiguous on both sides (max-size DMA
    # packets, minimal descriptor work in the critical lead-in)
    xT_d = nc.dram_tensor("xT", [128, CT, N], bf16, kind="ExternalInput")
    wqT_d = nc.dram_tensor("wqT", [128, CT, C], bf16, kind="ExternalInput")
    wkT_d = nc.dram_tensor("wkT", [128, CT, C], bf16, kind="ExternalInput")
    wvT_d = nc.dram_tensor("wvT", [128, CT, C], bf16, kind="ExternalInput")
    wpT_d = nc.dram_tensor("wpT", [128, CT, C], bf16, kind="ExternalInput")
    bpT_d = nc.dram_tensor("bpT", [128, CT], f32, kind="ExternalInput")
    # exp(attn_bias) packed per (pair, key-tile j, query-half nb):
    # [...,0:512] = even head, [...,512:1024] = odd head
    eb_d = nc.dram_tensor("ebPk", [NP, NT, 2, 128, 2 * F], bf16,
                          kind="ExternalInput")
    outT_d = nc.dram_tensor("outT", [C, N], bf16, kind="ExternalOutput")
    # softmax-sum scratch per normalization batch
    HA = 8
    sA_scr = nc.dram_tensor("sA_scr", [HA * N], bf16)
    sB1_scr = nc.dram_tensor("sB1_scr", [2 * N], bf16)
    sB2_scr = nc.dram_tensor("sB2_scr", [2 * N], bf16)
    rA_scr = nc.dram_tensor("rA_scr", [1, HA * N], bf16)
    rB1_scr = nc.dram_tensor("rB1_scr", [1, 2 * N], bf16)
    rB2_scr = nc.dram_tensor("rB2_scr", [1, 2 * N], bf16)

    with tile.TileContext(nc) as tc:
        with tc.tile_pool(name="persist", bufs=1) as pers:
            xTb = pers.tile([128, CT, N], bf16, tag="xT")
            wqb = pers.tile([128, CT, C], bf16, tag="wq")
            wkb = pers.tile([128, CT, C], bf16, tag="wk")
            wvb = pers.tile([128, CT, C], bf16, tag="wv")
            wpb = pers.tile([128, CT, C], bf16, tag="wp")
            bpb = pers.tile([128, CT], f32, tag="bp")
            # row 64 collects softmax sums (same partition as pv row 64)
            rba = pers.tile([128, H * N], bf16, tag="rba")
            qtb = pers.tile([128, CT, N], bf16, tag="qt")
            ktb = pers.tile([128, CT, N], bf16, tag="kt")
            vb = pers.tile([128, NT, H, D + 1], bf16, tag="v")
            atb = pers.tile([128, CT, N], bf16, tag="at")
            wsrc = pers.tile([128, 384], bf16, tag="wsrc")

            # fine-grained, need-ordered loads: the lead-in's ~5MB is
            # aggregate-bandwidth-bound, and pair 0's first QK needs ALL
            # of x (both query halves), so x gets the head of BOTH the
            # sync and scalar queues; weights follow in first-use order
            # fine-grained, need-ordered loads: the lead-in's ~5MB is
            # aggregate-bandwidth-bound, so what matters is that the
            # first consumer's slice lands first, not packet efficiency
            for q0 in range(0, N, 256):
                nc.sync.dma_start(xTb[:, :, q0:q0 + 256],
                                  xT_d[:, :, q0:q0 + 256])
            nc.scalar.dma_start(wvb[:, :, 0:512], wvT_d[:, :, 0:512])
            nc.gpsimd.dma_start(wkb[:, :, 0:128], wkT_d[:, :, 0:128])
            nc.gpsimd.dma_start(wkb[:, :, 128:C], wkT_d[:, :, 128:C])
            nc.scalar.dma_start(wqb[:, :, 0:128], wqT_d[:, :, 0:128])
            nc.scalar.dma_start(wqb[:, :, 128:C], wqT_d[:, :, 128:C])
            nc.scalar.dma_start(wvb[:, :, 512:C], wvT_d[:, :, 512:C])
            nc.scalar.dma_start(bpb, bpT_d[:])
            nc.scalar.dma_start(wpb, wpT_d[:])

            nc.vector.memset(wsrc, 1.0)
            nc.vector.memset(vb[:, :, :, D:D + 1], 1.0)

            with tc.tile_pool(name="ups", bufs=2, space="PSUM") as pU, \
                 tc.tile_pool(name="pvps", bufs=2, space="PSUM") as pPV, \
                 tc.tile_pool(name="fillps", bufs=2, space="PSUM") as pF, \
                 tc.tile_pool(name="ebb", bufs=4) as ebp, \
                 tc.tile_pool(name="vstagb", bufs=8) as vstagp, \
                 tc.tile_pool(name="nrmb", bufs=1) as nrm, \
                 tc.tile_pool(name="ptb", bufs=18) as ptp:

                eb_tiles = {}

                def eb_load(pr, j):
                    ebt = ebp.tile([128, 2, 2 * F], bf16, tag="eb")
                    nc.sync.dma_start(
                        ebt, eb_d[pr, j].rearrange("nb p q -> p nb q"))
                    eb_tiles[(pr, j)] = ebt

                def pe_prewarm(n):
                    """Dense burst of matmuls on the memset scratch: keeps
                    the PE busy (HAM clock gate at 8/8) from boot until x
                    lands (~17us) -- any >3.4us idle gap here re-throttles
                    the clock and the whole pre-pair runs at 1.2GHz."""
                    for _ in range(n):
                        ps = pF.tile([128, F], f32, tag="fill",
                                     name="prewarm")
                        nc.tensor.matmul(
                            ps[:, 0:256],
                            lhsT=wsrc[:, 0:128],
                            rhs=wsrc[:, 128:384],
                            start=True, stop=True,
                        )

                def v_proj(h0, nh, nt):
                    """V projection for heads [h0, h0+nh) at key-tile nt."""
                    f0, fw = h0 * D, nh * D
                    ps = pF.tile([128, F], f32, tag="fill")
                    for ci in range(CT):
                        nc.tensor.matmul(
                            ps[:, :fw],
                            lhsT=xTb[:, ci, nt * 128:(nt + 1) * 128],
                            rhs=wvb[:, ci, f0:f0 + fw],
                            start=(ci == 0),
                            stop=(ci == CT - 1),
                        )
                    nc.vector.tensor_copy(
                        vb[:, nt, h0:h0 + nh, 0:D],
                        ps[:, :fw].rearrange("p (h d) -> p h d", d=D),
                    )

                def pe_warm():
                    """Redundant 6-matmul group (recomputes k-projection
                    tile 0 into a dead PSUM tile, never read).  Emitted in
                    filler-starved stretches so the PE's activity monitor
                    does not re-throttle the clock (K=4/8) on micro-idle."""
                    ps = pF.tile([128, F], f32, tag="fill", name="warm")
                    for ci in range(CT):
                        nc.tensor.matmul(
                            ps,
                            lhsT=wkb[:, ci, 0:128],
                            rhs=xTb[:, ci, 0:F],
                            start=(ci == 0),
                            stop=(ci == CT - 1),
                        )

                def kq_sub(which, cot, nb):
                    wb, dst = (wkb, ktb) if which == "k" else (wqb, qtb)
                    ps = pF.tile([128, F], f32, tag="fill")
                    for ci in range(CT):
                        nc.tensor.matmul(
                            ps,
                            lhsT=wb[:, ci, cot * 128:(cot + 1) * 128],
                            rhs=xTb[:, ci, nb * F:(nb + 1) * F],
                            start=(ci == 0),
                            stop=(ci == CT - 1),
                        )
                    nc.vector.tensor_copy(
                        dst[:, cot, nb * F:(nb + 1) * F], ps)

                def kq_ct(cot):
                    for which in ("k", "q"):
                        for nb in range(2):
                            kq_sub(which, cot, nb)

                def qk_pair(pr, j, nb):
                    """Row-tiled QK^T for both heads of the pair: even head
                    on PE rows 0:64, odd head on rows 64:128, concurrent."""
                    ksl = slice(j * 128, (j + 1) * 128)
                    qsl = slice(nb * F, (nb + 1) * F)
                    sp = pU.tile([128, 2 * F], f32, tag="ps",
                                 name=f"sp_{pr}_{j}_{nb}")
                    nc.tensor.matmul(
                        sp[:, 0:F],
                        lhsT=ktb[0:64, pr, ksl],
                        rhs=qtb[0:64, pr, qsl],
                        start=True, stop=True,
                    )
                    nc.tensor.matmul(
                        sp[:, F:2 * F],
                        lhsT=ktb[64:128, pr, ksl],
                        rhs=qtb[64:128, pr, qsl],
                        start=True, stop=True,
                    )
                    return sp

                def pv_evac(pr, hl, pv_nb):
                    """Evacuate one head's two PV chains; softmax-sum row
                    64 goes straight to the DRAM scratch its norm batch
                    reads."""
                    h = 2 * pr + hl
                    s_scr, hh = (
                        (sA_scr, h) if h < HA
                        else (sB1_scr, h - HA) if h < HA + 2
                        else (sB2_scr, h - HA - 2))
                    for nb in range(2):
                        qsl = slice(nb * F, (nb + 1) * F)
                        vstag = vstagp.tile([D + 1, F], bf16, tag="vstag")
                        nc.vector.tensor_copy(vstag, pv_nb[nb])
                        nc.gpsimd.dma_start(
                            atb[64 * hl:64 * hl + 64, pr, qsl],
                            vstag[0:D, :])
                        nc.gpsimd.dma_start(
                            s_scr[hh * N + nb * F:hh * N + (nb + 1) * F],
                            vstag[D:D + 1, :])

                def attn_pair(pr, fillers, prev_tail=None):
                    """Attention for heads (2*pr, 2*pr+1).

                    Emission order per j: exp/mul for j, then QK for j+1
                    (so the PE never head-of-line blocks on the softmax
                    chain), then the even head's PV matmuls for j, then
                    fillers.  The odd head's PV chains run as one dense
                    16-matmul block on the SBUF-buffered pt tiles --
                    DEFERRED into the start of the NEXT pair (returned as
                    a tail closure, emitted right after the next pair's
                    first QK), so the next pair's exp stream starts on ACT
                    while the PE runs the block: the pair-boundary ACT
                    bubble (~3us) disappears.
                    """
                    sps = [qk_pair(pr, 0, nb) for nb in range(2)]
                    if prev_tail is not None:
                        prev_tail[0]()
                    pv0 = [pPV.tile([D + 1, F], f32, tag="pv",
                                    name=f"pv_{pr}_0_{nb}")
                           for nb in range(2)]
                    pts_all = []
                    for j in range(NT):
                        ebt = eb_tiles.pop((pr, j))
                        pts = []
                        for nb in range(2):
                            pt = ptp.tile([128, 2 * F], bf16, tag="pt",
                                          name=f"pt_{pr}_{j}_{nb}")
                            nc.scalar.activation(
                                pt, sps[nb], AF.Exp, scale=0.125)
                            nc.vector.tensor_tensor(
                                pt, pt, ebt[:, nb, :], ALU.mult)
                            pts.append(pt)
                        pts_all.append(pts)
                        if j + 1 < NT:
                            sps = [qk_pair(pr, j + 1, nb) for nb in range(2)]
                        if j == 0 and prev_tail is not None:
                            prev_tail[1]()
                        for nb in range(2):
                            nc.tensor.matmul(
                                pv0[nb],
                                lhsT=vb[:, j, 2 * pr, :],
                                rhs=pts[nb][:, 0:F],
                                start=(j == 0),
                                stop=(j == NT - 1),
                            )
                        for fn in fillers.get(j, ()):
                            fn()

                    pv1 = []

                    def tail1():
                        pv_evac(pr, 0, pv0)
                        pv1.extend(
                            pPV.tile([D + 1, F], f32, tag="pv",
                                     name=f"pv_{pr}_1_{nb}")
                            for nb in range(2))
                        for j in range(NT // 2):
                            for nb in range(2):
                                nc.tensor.matmul(
                                    pv1[nb],
                                    lhsT=vb[:, j, 2 * pr + 1, :],
                                    rhs=pts_all[j][nb][:, F:2 * F],
                                    start=(j == 0),
                                    stop=False,
                                )

                    def tail2():
                        for j in range(NT // 2, NT):
                            for nb in range(2):
                                nc.tensor.matmul(
                                    pv1[nb],
                                    lhsT=vb[:, j, 2 * pr + 1, :],
                                    rhs=pts_all[j][nb][:, F:2 * F],
                                    start=False,
                                    stop=(j == NT - 1),
                                )
                        pv_evac(pr, 1, pv1)
                    return tail1, tail2

                _NB = [
                    (0, HA, sA_scr, rA_scr),
                    (HA, 2, sB1_scr, rB1_scr),
                    (HA + 2, 2, sB2_scr, rB2_scr),
                ]
                nb_sb = {}

                def norm_load(batch):
                    """Reload the batch's softmax sums.  On the scalar
                    queue: three small issues total, so it neither waits
                    behind the sync queue's exp(bias) backlog nor stalls
                    the gpsimd evacuation stream."""
                    h0, nh, s_scr, r_scr = _NB[batch]
                    cols = nh * N // 128
                    sb = nrm.tile([128, HA * N // 128], bf16, tag="sb")
                    nc.scalar.dma_start(
                        sb[:, :cols],
                        s_scr[:].rearrange("(p f) -> p f", p=128))
                    nb_sb[batch] = sb

                def norm_fin(batch):
                    """Reciprocal + store + per-head broadcasts.  Emitted
                    a few filler slots after norm_load so the DVE
                    reciprocal never head-of-line blocks the DVE queue
                    waiting for the sums to land."""
                    h0, nh, s_scr, r_scr = _NB[batch]
                    cols = nh * N // 128
                    sb = nb_sb.pop(batch)
                    rc32 = nrm.tile([128, HA * N // 128], f32, tag="rc32")
                    nc.vector.reciprocal(rc32[:, :cols], sb[:, :cols])
                    rcb = nrm.tile([128, HA * N // 128], bf16, tag="rcb")
                    nc.vector.tensor_copy(rcb[:, :cols], rc32[:, :cols])
                    nc.sync.dma_start(
                        r_scr[0, :].rearrange("(p f) -> p f", p=128),
                        rcb[:, :cols])

                def norm_bcast(h, eng=None):
                    """Broadcast head h's reciprocals into the 64
                    partitions its norm-multiply reads.  Emitted in the
                    post-pair window (sync queue is idle there) -- doing
                    this mid-pair delayed the exp(bias) prefetch by the
                    ~1MB of broadcast traffic and stretched pairs 4-5."""
                    for b0, nh, _, r_scr in _NB:
                        if b0 <= h < b0 + nh:
                            break
                    po = 64 * (h % 2)
                    (eng or nc.sync).dma_start(
                        rba[po:po + 64, h * N:(h + 1) * N],
                        r_scr[:, (h - b0) * N:(h - b0 + 1) * N]
                        .to_broadcast([64, N]))

                def norm_mul(h, engine=None):
                    """Normalize-multiply one head's atb slice.  Always on
                    DVE: gpsimd compute both locks the shared SBUF port
                    pair and starves its own DMA descriptor generation,
                    backing up the evacuation queue."""
                    ct, po = h // 2, 64 * (h % 2)
                    sl = atb[po:po + 64, ct, :]
                    eng = engine or nc.vector
                    eng.tensor_tensor(
                        sl, sl, rba[po:po + 64, h * N:(h + 1) * N], ALU.mult)

                # ---- emission schedule --------------------------------
                EB_DEPTH = 4
                for t in range(EB_DEPTH):
                    eb_load(0, t)
                pe_prewarm(20)
                for nt in range(NT):
                    v_proj(0, 8, nt)
                kq_ct(0)

                # fillers per pair: kq projection for pair+1 (incl. pair
                # 1's k inside pair 0), rolling EB prefetch, V projection
                # for heads 8:12 during pairs 2-3, per-pair normalization
                # one pair behind, PE warm-keepers in the filler-starved
                # pairs 4-5.
                prev_tail = None
                for pr in range(NP):
                    fill = {j: [] for j in range(NT)}
                    for j in range(NT):
                        t = pr * NT + j + EB_DEPTH
                        if t < NP * NT:
                            fill[j].append(
                                lambda a=t // NT, b=t % NT: eb_load(a, b))
                    nxt = pr + 1
                    if nxt < NP:
                        fill[1].append(lambda n=nxt: kq_sub("k", n, 0))
                        fill[2].append(lambda n=nxt: kq_sub("k", n, 1))
                        fill[4].append(lambda n=nxt: kq_sub("q", n, 0))
                        fill[5].append(lambda n=nxt: kq_sub("q", n, 1))
                    if pr == 2:
                        for nt in range(4):
                            fill[6].append(lambda t=nt: v_proj(8, 4, t))
                    if pr == 3:
                        for nt in range(4, NT):
                            fill[6].append(lambda t=nt: v_proj(8, 4, t))
                    if pr == 4:
                        # heads 0..7 sums complete after pair 3's evac
                        # (whose deferred tail runs at this pair's start);
                        # only the cheap reciprocal chains run inside the
                        # pairs -- all norm-multiplies are deferred to the
                        # output-projection phase where DVE is idle
                        fill[0].append(lambda: norm_load(0))
                        fill[2].append(lambda: norm_fin(0))
                    if pr == 5:
                        fill[0].append(lambda: norm_load(1))
                        fill[2].append(lambda: norm_fin(1))
                        fill[3].append(pe_warm)
                        # heads 0-9 reciprocal broadcasts: their data is
                        # ready (batch 0 stored in pair 4, batch 1 by
                        # fill[2]) and pair 5's eb prefetch is done by
                        # j3, so the sync queue is free -- landing them
                        # here lets every norm-multiply fire immediately
                        # at tail start instead of gating the projection
                        for j, hs in ((4, (0, 1, 2)), (5, (3, 4, 5)),
                                      (6, (6, 7)), (7, (8, 9))):
                            for h in hs:
                                fill[j].append(lambda hh=h: norm_bcast(hh))
                    prev_tail = attn_pair(pr, fill, prev_tail)

                # tail pipeline: pair 5's deferred PV block + evac runs
                # on PE while the reciprocal broadcasts stream on the
                # idle sync queue and the norm-multiplies drain on DVE;
                # batch 2's reciprocal is emitted mid-stream so it never
                # head-of-line blocks the DVE queue
                prev_tail[0]()
                prev_tail[1]()
                norm_load(2)
                for h in range(4):
                    norm_mul(h)
                # batch-2's reciprocal is emitted mid-mul-stream: late
                # enough that the DVE reciprocal has its data, early
                # enough that the h10/11 chain doesn't gate the final
                # projection tiles
                norm_fin(2)
                for h in range(4, 10):
                    norm_mul(h)

            # ---- output projection ------------------------------------
            # [128, 512] granularity (one PSUM bank per tile, 8 in
            # flight).  Pass 1 (ci 0..4: heads 0..9, normalized by
            # mid-pair-5) for the first 8 tiles is emitted BEFORE the
            # heads-10/11 normalize-multiplies; each tile then finishes
            # with the ci=5 matmul, a small ACT bias-add, and a store
            # alternating the sync/scalar queues.  Emitting more than 8
            # pass-1 tiles would deadlock the PE FIFO on pool-slot reuse.
            with tc.tile_pool(name="ops", bufs=8, space="PSUM") as pC, \
                 tc.tile_pool(name="otb", bufs=3) as otp:

                def oproj_mm(ps, cot, nb, ci, start):
                    nc.tensor.matmul(
                        ps,
                        lhsT=wpb[:, ci, cot * 128:(cot + 1) * 128],
                        rhs=atb[:, ci, nb * F:(nb + 1) * F],
                        start=start,
                        stop=False,
                    )

                def oproj_fin(psn, cot):
                    """Finish both query halves of one output-channel
                    tile: ci=5 matmuls, bias-add ACTs, ONE merged store
                    (12 -> 6 store issues: the serialized per-store issue
                    cost on the sync queue dominated the output drain)."""
                    ot = otp.tile([128, 2 * F], bf16, tag="ot",
                                  name=f"ot_{cot}")
                    for nb in range(2):
                        nc.tensor.matmul(
                            psn[nb],
                            lhsT=wpb[:, CT - 1, cot * 128:(cot + 1) * 128],
                            rhs=atb[:, CT - 1, nb * F:(nb + 1) * F],
                            start=False, stop=True,
                        )
                    for nb in range(2):
                        nc.scalar.activation(
                            ot[:, nb * F:(nb + 1) * F], psn[nb],
                            AF.Identity, bias=bpb[:, cot:cot + 1])
                    nc.sync.dma_start(
                        outT_d[cot * 128:(cot + 1) * 128, :], ot)

                # pass 1 (ci 0..4, heads 0..9) runs ci-MAJOR so the first
                # 16 matmuls only need heads 0-3 normalized: the tile
                # accumulation pipelines with the norm-multiply stream
                # still draining on DVE
                pss = {(cot, nb): pC.tile([128, F], f32, tag="o",
                                          name=f"po_{cot}_{nb}")
                       for cot in range(4) for nb in range(2)}
                for i, ci in enumerate(range(CT - 1)):
                    for (cot, nb), ps in pss.items():
                        oproj_mm(ps, cot, nb, ci, start=(i == 0))
                norm_bcast(10)
                norm_bcast(11)
                norm_mul(10)
                norm_mul(11)
                for cot in range(4):
                    oproj_fin([pss[(cot, 0)], pss[(cot, 1)]], cot)
                for cot in range(4, CT):
                    psn = [pC.tile([128, F], f32, tag="o",
                                   name=f"po_{cot}_{nb}")
                           for nb in range(2)]
                    for i, ci in enumerate(range(CT - 1)):
                        for nb in range(2):
                            oproj_mm(psn[nb], cot, nb, ci, start=(i == 0))
                    oproj_fin(psn, cot)

    nc.compile()
    return nc


def _get_nc():
    if "nc" not in _cache:
        _cache["nc"] = _build()
    return _cache["nc"]


def _pack(mT):
    """[C, F] -> [128, CT, F] with row ci*128+p landing at [p, ci]."""
    f = mT.shape[1]
    return np.ascontiguousarray(
        mT.reshape(CT, 128, f).transpose(1, 0, 2)).astype(BF16)


def prep_in_maps(x, attn_bias, Wq, Wk, Wv, Wp, bp):
    """Host-side sharding + layout prep (transposes/casts/exp of bias)."""
    wqT = _pack(Wq.T)
    wkT = _pack(Wk.T)
    wvT = _pack(Wv.T)
    wpT = _pack(Wp.T)
    bpT = np.ascontiguousarray(bp.astype(np.float32).reshape(CT, 128).T)
    # exp(bias)^T packed per (pair, key-tile, query-half): see kernel docstr
    E = np.exp(attn_bias[0].astype(np.float32)).transpose(0, 2, 1)
    E = np.ascontiguousarray(E).reshape(H, NT, 128, 2, F)
    ebPk = np.empty((NP, NT, 2, 128, 2 * F), dtype=np.float32)
    for pr in range(NP):
        ebPk[pr, :, :, :, 0:F] = E[2 * pr].transpose(0, 2, 1, 3)
        ebPk[pr, :, :, :, F:2 * F] = E[2 * pr + 1].transpose(0, 2, 1, 3)
    ebPk = ebPk.astype(BF16)
    in_maps = []
    for b in range(B):
        in_maps.append({
            "xT": _pack(np.asarray(x[b]).T),
            "wqT": wqT, "wkT": wkT, "wvT": wvT, "wpT": wpT,
            "bpT": bpT, "ebPk": ebPk,
        })
    return in_maps


def run(in_maps, trace=False, **kw):
    from concourse.bass_utils import run_bass_kernel_spmd

    nc = _get_nc()
    return run_bass_kernel_spmd(
        nc, in_maps, core_ids=list(range(B)), trace=trace, **kw
    )


def kernel(x, attn_bias, Wq, Wk, Wv, Wp, bp):
    res = run(prep_in_maps(x, attn_bias, Wq, Wk, Wv, Wp, bp))
    out = np.stack(
        [res.results[b]["outT"].T for b in range(B)]
    ).astype(np.float32)
    return out
